# revision 11
# baseline (speedup 1.0000x reference)
"""Trainium2 Bass kernel for nn_ANPToolEncoder (sparse attention encoder).

Sharding: M=64 context groups split across 8 NeuronCores (8 groups each);
the whole network is embarrassingly parallel in M — each core computes
out[:, m_shard, :] and the host concatenates. No collectives.

Layout convention on-chip: activations are kept "feat-major" ([feature
partitions, token free]) so every matmul contraction runs over the
partition axis; softmax denominators for self-attention are computed with
ones-vector matmuls on the PE; the cross-attention softmax denominator is
never computed at all (the final LayerNorm is invariant to per-row scale).
LayerNorm-then-linear (context LN -> V projection) is folded into the V
matmul via host-side weight scaling (wv*g) plus per-token istd/mu
correction terms.
"""

import os
import sys
import numpy as np

for _p in ("/opt/trn_rl_repo", "/root/.axon_site/_ro/trn_rl_repo"):
    if os.path.isdir(_p) and _p not in sys.path:
        sys.path.append(_p)

from concourse import bass, bacc, tile, mybir  # noqa: E402
from concourse.bass_utils import run_bass_kernel_spmd  # noqa: E402

B, M, C, DX, H, NH = 256, 64, 256, 512, 512, 8
DH = H // NH
EPS = 1e-5
N_CORES = 8
MLOC = M // N_CORES          # 8 context groups per core
KC = H // 128                # 4 feature chunks of 128
CC = C // 128                # 2 token chunks per group
BC = B // 128                # 2 query chunks

F32 = mybir.dt.float32
BF16 = mybir.dt.bfloat16
ACT = mybir.ActivationFunctionType
ALU = mybir.AluOpType

# compute dtype for matmul-feeding SBUF tensors ("f32" or "bf16")
COMPUTE_DT = os.environ.get("KERNEL_DT", "bf16")


def _np_dt(dt_str):
    if dt_str == "bf16":
        import ml_dtypes
        return np.dtype(ml_dtypes.bfloat16)
    return np.dtype(np.float32)


def _prep(inp):
    """Host-side weight folds + feat-major layouts. Returns (shared_map, per-core key)."""
    f = {k: np.asarray(v, np.float32) for k, v in inp.items()}
    w1 = f["cp_w1"]                                   # [H, DX+2]
    inw, inb = f["in_w"], f["in_b"]
    bq, bk, bv = inb[:H], inb[H:2*H], inb[2*H:]

    def chunkT(w):      # [out, in] -> [128, in/128, out]  (feat-major, k-chunked)
        wT = w.T                                      # [in, out]
        return np.ascontiguousarray(
            wT.reshape(wT.shape[0] // 128, 128, wT.shape[1]).transpose(1, 0, 2))

    def chunkv(v):      # [H] -> [128, KC]
        return np.ascontiguousarray(v.reshape(KC, 128).T)

    sH = 1.0 / np.sqrt(H)
    shared = {
        "w1kT": chunkT(w1[:, :DX]),                   # [128,4,512]
        "w1gT": np.ascontiguousarray(w1[:, DX:DX+2].T),  # [2,512]
        "b1":   chunkv(f["cp_b1"]),
        "w2T":  chunkT(f["cp_w2"]),
        "b2":   chunkv(f["cp_b2"]),
        "inwT": chunkT(inw),                          # [128,4,1536]
        "bq8":  chunkv(bq / np.sqrt(DH)),
        "bk":   chunkv(bk),
        "outwT": chunkT(f["out_w"]),
        "outbrow": (f["out_b"] + f["out_w"] @ bv)[None, :],   # [1,512]
        "wvgT": chunkT(f["wv_w"] * f["lnc_g"][None, :]),
        "kgb":  np.broadcast_to(f["wv_w"] @ f["lnc_g"], (128, H)).copy(),
        "kv2b": np.broadcast_to(f["wv_w"] @ f["lnc_b"] + f["wv_b"], (128, H)).copy(),
        "wkT":  chunkT(f["wk_w"]),
        "wkb":  chunkv(f["wk_b"]),
        "wqT":  chunkT(f["wq_w"]),
        "wqbs": chunkv(f["wq_b"] * sH),
        "lnogb": np.broadcast_to(f["lno_g"], (128, H)).copy(),
        "lnobb": np.broadcast_to(f["lno_b"], (128, H)).copy(),
        # query embed feat-major: [128, 4, 256]
        "qet": np.ascontiguousarray(
            f["query_embed"].T.reshape(KC, 128, B).transpose(1, 0, 2)),
    }

    # per-core X feat-major [128, 5, MLOC*C]: chunks 0-3 img dims, chunk 4
    # rows 0/1 = gt/pred (rest zero, never read)
    img, gt, pr = f["ctx_img_feat"], f["ctx_gt"], f["ctx_pred"]
    xts = []
    for ci in range(N_CORES):
        gs = slice(ci * MLOC, (ci + 1) * MLOC)
        xi = img[gs].reshape(MLOC * C, DX).T          # [512, 2048]
        xt = np.zeros((128, 5, MLOC * C), np.float32)
        xt[:, :4, :] = xi.reshape(4, 128, MLOC * C).transpose(1, 0, 2)
        xt[0, 4, :] = gt[gs].reshape(-1)
        xt[1, 4, :] = pr[gs].reshape(-1)
        xts.append(xt)
    return shared, xts


# names of DT-typed (matmul-feeding) params; everything else stays f32
_DT_PARAMS = {"w1kT", "w1gT", "w2T", "inwT", "outwT", "outbrow", "wvgT",
              "kgb", "kv2b", "wkT", "wqT", "lnogb", "lnobb", "qet", "xt"}

_SHAPES = {
    "xt":    [128, 5, MLOC * C],
    "qet":   [128, KC, B],
    "w1kT":  [128, KC, H], "w1gT": [2, H], "b1": [128, KC],
    "w2T":   [128, KC, H], "b2": [128, KC],
    "inwT":  [128, KC, 3 * H], "bq8": [128, KC], "bk": [128, KC],
    "outwT": [128, KC, H], "outbrow": [1, H],
    "wvgT":  [128, KC, H], "kgb": [128, H], "kv2b": [128, H],
    "wkT":   [128, KC, H], "wkb": [128, KC],
    "wqT":   [128, KC, H], "wqbs": [128, KC],
    "lnogb": [128, H], "lnobb": [128, H],
}


def _build(dt_str, skip_kv2, skip_lnog, skip_lnob, stage=99):
    DT = BF16 if dt_str == "bf16" else F32
    nc = bacc.Bacc("TRN2", target_bir_lowering=False, debug=False,
                   num_devices=N_CORES)

    P = {}
    for name, shp in _SHAPES.items():
        pdt = DT if name in _DT_PARAMS else F32
        P[name] = nc.declare_dram_parameter(name, shp, pdt, isOutput=False)
    out_ext = nc.declare_dram_parameter("out", [B, MLOC, H], F32, isOutput=True)

    with tile.TileContext(nc) as tc:
        with tc.tile_pool(name="wt", bufs=1) as wt, \
             tc.tile_pool(name="wk", bufs=1) as wk, \
             tc.tile_pool(name="sm", bufs=2) as sm, \
             tc.tile_pool(name="ps", bufs=6, space="PSUM") as psp, \
             tc.tile_pool(name="st", bufs=2, space="PSUM") as stp:

            # ---- load weights / consts ----
            unused = set()
            if skip_kv2:
                unused.add("kv2b")
            if skip_lnog:
                unused.add("lnogb")
            if skip_lnob:
                unused.add("lnobb")
            W = {}
            for name in _SHAPES:
                if name in unused:
                    continue
                pdt = DT if name in _DT_PARAMS else F32
                t = wt.tile(_SHAPES[name], pdt, tag=name)
                nc.sync.dma_start(out=t[...], in_=P[name][...])
                W[name] = t
            ones128 = wt.tile([128, 1], DT, tag="ones128")
            nc.vector.memset(ones128[:], 1.0)
            onesrow = wt.tile([1, C], DT, tag="onesrow")
            nc.vector.memset(onesrow[:], 1.0)
            onesbc = wt.tile([128, 64], F32, tag="onesbc")
            nc.vector.memset(onesbc[:], 1.0)
            epsc = wt.tile([128, 1], F32, tag="epsc")
            nc.vector.memset(epsc[:], EPS)

            def mm_chain(ps_ap, pairs):
                """Accumulating matmul chain: pairs = [(lhsT, rhs), ...]."""
                n = len(pairs)
                for i, (l, r) in enumerate(pairs):
                    nc.tensor.matmul(ps_ap, l, r, start=(i == 0), stop=(i == n - 1),
                                     skip_group_check=True)

            # ---- Q projection (once, replicated) ----
            QT = wt.tile([128, KC, B], DT, tag="QT")
            if stage >= 1:
                for hc in range(KC):
                    ps = psp.tile([128, B], F32, tag="ps")
                    mm_chain(ps[...], [(W["wqT"][:, k, hc*128:(hc+1)*128],
                                        W["qet"][:, k, :]) for k in range(KC)])
                    nc.scalar.activation(QT[:, hc, :], ps[...], ACT.Identity,
                                         bias=W["wqbs"][:, hc:hc+1],
                                         scale=1.0 / float(np.sqrt(H)))

            # ---- per-group pipeline ----
            for g in range(MLOC):
                xg = slice(g * C, (g + 1) * C)

                if stage < 2:
                    continue
                # MLP1: h1 = relu(W1 @ x + b1)   feat-major [128, 4, 256]
                h1 = wk.tile([128, KC, C], DT, tag="h1")
                for hc in range(KC):
                    ps = psp.tile([128, C], F32, tag="ps")
                    pairs = [(W["w1kT"][:, k, hc*128:(hc+1)*128],
                              W["xt"][:, k, xg]) for k in range(4)]
                    pairs.append((W["w1gT"][0:2, hc*128:(hc+1)*128],
                                  W["xt"][0:2, 4, xg]))
                    mm_chain(ps[...], pairs)
                    nc.scalar.activation(h1[:, hc, :], ps[...], ACT.Relu,
                                         bias=W["b1"][:, hc:hc+1])

                if stage < 3:
                    continue
                # MLP2: ctx = W2 @ h1 + b2
                ctx = wk.tile([128, KC, C], DT, tag="ctx")
                for hc in range(KC):
                    ps = psp.tile([128, C], F32, tag="ps")
                    mm_chain(ps[...], [(W["w2T"][:, k, hc*128:(hc+1)*128],
                                        h1[:, k, :]) for k in range(KC)])
                    nc.scalar.activation(ctx[:, hc, :], ps[...], ACT.Identity,
                                         bias=W["b2"][:, hc:hc+1])

                if stage < 4:
                    continue
                # in-proj q,k (feat-major, q pre-scaled by 1/sqrt(DH))
                qk = wk.tile([128, 2 * KC, C], DT, tag="qk")
                for jc in range(2 * KC):
                    ps = psp.tile([128, C], F32, tag="ps")
                    mm_chain(ps[...], [(W["inwT"][:, k, jc*128:(jc+1)*128],
                                        ctx[:, k, :]) for k in range(KC)])
                    if jc < KC:
                        nc.scalar.activation(qk[:, jc, :], ps[...], ACT.Identity,
                                             bias=W["bq8"][:, jc:jc+1],
                                             scale=1.0 / float(np.sqrt(DH)))
                    else:
                        nc.scalar.activation(qk[:, jc, :], ps[...], ACT.Identity,
                                             bias=W["bk"][:, jc-KC:jc-KC+1])

                if stage < 5:
                    continue
                # in-proj v token-major [c, j] (bias folded into out-proj bias)
                vtok = wk.tile([128, CC, H], DT, tag="vtok")
                for cc in range(CC):
                    ps = psp.tile([128, H], F32, tag="ps")
                    mm_chain(ps[...], [(ctx[:, k, cc*128:(cc+1)*128],
                                        W["inwT"][:, k, 2*H:3*H]) for k in range(KC)])
                    nc.scalar.activation(vtok[:, cc, :], ps[...], ACT.Copy)

                if stage < 6:
                    continue
                # self-attention: scores_T = K^T Q per head; exp; denominators
                # via ones-matmul; sa0 feat-major; divide via K=1 bcast matmul
                # head pairs share a base partition (row group) so that no
                # PSUM bank is ever written from two PE row groups at once
                PAIRS = [(0, 2), (4, 6), (1, 3), (5, 7)]
                den_ps = psp.tile([128, 2 * C], F32, tag="ps")
                PTs = []
                for p, pair in enumerate(PAIRS):
                    PT = wk.tile([128, CC, 2 * C], DT, tag=f"PT{p}")
                    PTs.append(PT)
                    for kc in range(CC):
                        ps = psp.tile([128, 2 * C], F32, tag="ps")
                        for hh, h in enumerate(pair):
                            off = 64 * (h % 2)
                            jslot = h // 2
                            lhsT = qk[off:off+64, KC + jslot, kc*128:(kc+1)*128]
                            rhs = qk[off:off+64, jslot, :]
                            nc.tensor.matmul(ps[:, hh*C:(hh+1)*C], lhsT, rhs,
                                             start=True, stop=True,
                                             skip_group_check=True)
                        nc.scalar.activation(PT[:, kc, :], ps[...], ACT.Exp)
                    if stage < 7:
                        continue
                    # denominators for this pair -> row 32p of den_ps
                    for kc in range(CC):
                        nc.tensor.matmul(den_ps[32*p:32*p+1, :], ones128[:, 0:1],
                                         PT[:, kc, :], start=(kc == 0),
                                         stop=(kc == CC - 1),
                                         skip_group_check=True,
                                         tile_position=(0, 32 * p))
                if stage < 8:
                    continue
                den_sb = wk.tile([128, 2 * C], F32, tag="den")
                nc.scalar.activation(den_sb[...], den_ps[...], ACT.Copy)
                inv_sb = wk.tile([128, 2 * C], F32, tag="inv")
                nc.vector.reciprocal(inv_sb[...], den_sb[...])

                if stage < 9:
                    continue
                saT = wk.tile([128, KC, C], DT, tag="saT")
                for p, pair in enumerate(PAIRS):
                    PT = PTs[p]
                    sa0 = psp.tile([128, C], F32, tag="ps")
                    bc = psp.tile([128, C], F32, tag="ps")
                    for hh, h in enumerate(pair):
                        mm_chain(sa0[64*hh:64*hh+64, :],
                                 [(vtok[:, kc, 64*h:64*h+64], PT[:, kc, hh*C:(hh+1)*C])
                                  for kc in range(CC)])
                        nc.tensor.matmul(bc[64*hh:64*hh+64, :],
                                         onesbc[32*p:32*p+1, :],
                                         inv_sb[32*p:32*p+1, hh*C:(hh+1)*C],
                                         start=True, stop=True,
                                         skip_group_check=True,
                                         tile_position=(32 * p, 64 * hh))
                    bc_sb = sm.tile([128, C], F32, tag="bc")
                    nc.vector.tensor_copy(bc_sb[...], bc[...])
                    # head h lives at sa_T rows [64h, 64h+64): chunk h//2,
                    # partition offset 64*(h%2)
                    for hh, h in enumerate(pair):
                        o = 64 * (h % 2)
                        nc.vector.tensor_tensor(saT[o:o+64, h // 2, :],
                                                sa0[64*hh:64*hh+64, :],
                                                bc_sb[64*hh:64*hh+64, :],
                                                ALU.mult)

                if stage < 10:
                    continue
                # out-proj + residual: r = ctx + outw @ sa + outb_eff
                rT = wk.tile([128, KC, C], DT, tag="rT")
                r2T = wk.tile([128, KC, C], DT, tag="r2T")
                for hc in range(KC):
                    ps = psp.tile([128, C], F32, tag="ps")
                    pairs = [(W["outwT"][:, k, hc*128:(hc+1)*128], saT[:, k, :])
                             for k in range(KC)]
                    pairs.append((W["outbrow"][0:1, hc*128:(hc+1)*128],
                                  onesrow[0:1, :]))
                    mm_chain(ps[...], pairs)
                    nc.vector.tensor_tensor(rT[:, hc, :], ps[...], ctx[:, hc, :],
                                            ALU.add)
                    nc.scalar.activation(r2T[:, hc, :], rT[:, hc, :], ACT.Square)

                if stage < 11:
                    continue
                # context-LN stats via ones-matmuls (per token chunk cc)
                istds, ts = [], []
                for cc in range(CC):
                    stat = stp.tile([128, 2], F32, tag="st")
                    mm_chain(stat[:, 0:1], [(rT[:, k, cc*128:(cc+1)*128],
                                             ones128[:, 0:1]) for k in range(KC)])
                    mm_chain(stat[:, 1:2], [(r2T[:, k, cc*128:(cc+1)*128],
                                             ones128[:, 0:1]) for k in range(KC)])
                    mu = sm.tile([128, 1], F32, tag="mu")
                    nc.vector.tensor_scalar(mu[...], stat[:, 0:1], 1.0 / H, None,
                                            ALU.mult)
                    musq = sm.tile([128, 1], F32, tag="musq")
                    nc.scalar.activation(musq[...], mu[...], ACT.Square)
                    s1 = sm.tile([128, 1], F32, tag="s1")
                    nc.vector.tensor_scalar(s1[...], stat[:, 1:2], 1.0 / H, EPS,
                                            ALU.mult, ALU.add)
                    vpe = sm.tile([128, 1], F32, tag="vpe")
                    nc.vector.tensor_tensor(vpe[...], s1[...], musq[...],
                                            ALU.subtract)
                    lnv = sm.tile([128, 1], F32, tag="lnv")
                    nc.scalar.activation(lnv[...], vpe[...], ACT.Ln)
                    istd = sm.tile([128, 1], F32, tag="istd")
                    nc.scalar.activation(istd[...], lnv[...], ACT.Exp, scale=-0.5)
                    t_ = sm.tile([128, 1], F32, tag="t_")
                    nc.vector.tensor_scalar(t_[...], mu[...], istd[...], -1.0,
                                            ALU.mult, ALU.mult)
                    istds.append(istd)
                    ts.append(t_)

                if stage < 12:
                    continue
                # V = istd*(r @ wvg^T) + t*kg + kv2   (token-major [c, hv])
                V = wk.tile([128, CC, H], DT, tag="V")
                for cc in range(CC):
                    ps = psp.tile([128, H], F32, tag="ps")
                    mm_chain(ps[...], [(rT[:, k, cc*128:(cc+1)*128],
                                        W["wvgT"][:, k, :]) for k in range(KC)])
                    tmp1 = sm.tile([128, H], DT, tag="tmp1")
                    nc.scalar.activation(tmp1[...], ps[...], ACT.Copy,
                                         scale=istds[cc][...])
                    tmp2 = sm.tile([128, H], DT, tag="tmp2")
                    nc.vector.tensor_scalar(tmp2[...], W["kgb"][...], ts[cc][...],
                                            None, ALU.mult)
                    if skip_kv2:
                        nc.vector.tensor_tensor(V[:, cc, :], tmp1[...], tmp2[...],
                                                ALU.add)
                    else:
                        tmp3 = sm.tile([128, H], DT, tag="tmp3")
                        nc.vector.tensor_tensor(tmp3[...], tmp1[...], tmp2[...],
                                                ALU.add)
                        nc.vector.tensor_tensor(V[:, cc, :], tmp3[...],
                                                W["kv2b"][...], ALU.add)

                if stage < 13:
                    continue
                # cross-attn K projection (from raw img features)
                KT = wk.tile([128, KC, C], DT, tag="KT")
                for hc in range(KC):
                    ps = psp.tile([128, C], F32, tag="ps")
                    mm_chain(ps[...], [(W["wkT"][:, k, hc*128:(hc+1)*128],
                                        W["xt"][:, k, xg]) for k in range(4)])
                    nc.scalar.activation(KT[:, hc, :], ps[...], ACT.Identity,
                                         bias=W["wkb"][:, hc:hc+1])

                if stage < 14:
                    continue
                # logits_T [c, b] -> exp (denominator absorbed by final LN)
                PTc = wk.tile([128, CC, B], DT, tag="PTc")
                for cc in range(CC):
                    ps = psp.tile([128, B], F32, tag="ps")
                    mm_chain(ps[...], [(KT[:, k, cc*128:(cc+1)*128], QT[:, k, :])
                                       for k in range(KC)])
                    nc.scalar.activation(PTc[:, cc, :], ps[...], ACT.Exp)

                if stage < 15:
                    continue
                # z0 = P~^T V  [b, hv], then final LN (scale-invariant)
                for bc in range(BC):
                    z0 = psp.tile([128, H], F32, tag="ps")
                    mm_chain(z0[...], [(PTc[:, kc, bc*128:(bc+1)*128], V[:, kc, :])
                                       for kc in range(CC)])
                    bns = sm.tile([128, 6], F32, tag="bns")
                    nc.vector.bn_stats(bns[...], z0[...])
                    ms = sm.tile([128, 2], F32, tag="ms")
                    nc.vector.bn_aggr(ms[...], bns[...])
                    lnv = sm.tile([128, 1], F32, tag="lnvz")
                    nc.scalar.activation(lnv[...], ms[:, 1:2], ACT.Ln,
                                         bias=epsc[...])
                    istd = sm.tile([128, 1], F32, tag="istdz")
                    nc.scalar.activation(istd[...], lnv[...], ACT.Exp, scale=-0.5)
                    nmi = sm.tile([128, 1], F32, tag="nmi")
                    nc.vector.tensor_scalar(nmi[...], ms[:, 0:1], istd[...], -1.0,
                                            ALU.mult, ALU.mult)
                    if skip_lnog and skip_lnob:
                        o_sb = sm.tile([128, H], F32, tag="osb")
                        nc.scalar.activation(o_sb[...], z0[...], ACT.Identity,
                                             scale=istd[...], bias=nmi[...])
                    else:
                        t1 = sm.tile([128, H], F32, tag="t1")
                        nc.scalar.activation(t1[...], z0[...], ACT.Identity,
                                             scale=istd[...], bias=nmi[...])
                        o_sb = sm.tile([128, H], F32, tag="osb")
                        if skip_lnog:
                            nc.vector.tensor_tensor(o_sb[...], t1[...],
                                                    W["lnobb"][...], ALU.add)
                        elif skip_lnob:
                            nc.vector.tensor_tensor(o_sb[...], t1[...],
                                                    W["lnogb"][...], ALU.mult)
                        else:
                            t2 = sm.tile([128, H], F32, tag="t2")
                            nc.vector.tensor_tensor(t2[...], t1[...],
                                                    W["lnogb"][...], ALU.mult)
                            nc.vector.tensor_tensor(o_sb[...], t2[...],
                                                    W["lnobb"][...], ALU.add)
                    nc.sync.dma_start(out=out_ext[bc*128:(bc+1)*128, g, :],
                                      in_=o_sb[...])

    nc.finalize()
    return nc


_CACHE = {}


def _get_nc(key):
    if key not in _CACHE:
        _CACHE[key] = _build(*key)
    return _CACHE[key]


def kernel(**inputs):
    shared, xts = _prep(inputs)
    skip_kv2 = bool(np.all(shared["kv2b"] == 0.0))
    skip_lnog = bool(np.all(shared["lnogb"] == 1.0))
    skip_lnob = bool(np.all(shared["lnobb"] == 0.0))
    key = (COMPUTE_DT, skip_kv2, skip_lnog, skip_lnob)
    nc = _get_nc(key)

    np_dt = _np_dt(COMPUTE_DT)
    in_maps = []
    for ci in range(N_CORES):
        m = {}
        for name in _SHAPES:
            arr = xts[ci] if name == "xt" else shared[name]
            pdt = np_dt if name in _DT_PARAMS else np.dtype(np.float32)
            m[name] = np.ascontiguousarray(arr.astype(pdt))
        in_maps.append(m)

    res = run_bass_kernel_spmd(nc, in_maps, list(range(N_CORES)),
                               trace=bool(int(os.environ.get("KERNEL_TRACE", "0"))))
    kernel.last_results = res
    out = np.concatenate([res.results[ci]["out"] for ci in range(N_CORES)], axis=1)
    return out.astype(np.float32)
